# revision 13
# baseline (speedup 1.0000x reference)
"""Trainium2 Bass kernel for Attention2D (LN -> MHA -> out-proj -> residual).

Sharding: data-parallel over batch — 8 batches, one per NeuronCore.

Per-core layout strategy (all per-batch tensors [C, S] = [512, 1024],
channel-major, which is x's natural layout):

  1. Stats: PE-transpose x tiles -> bn_stats/bn_aggr per seq position.
     LayerNorm is folded into the projections (affine fold):
       tn = (x - mu)*rstd*g + b
       Q^T = Wg^T @ (x*rstd) + [c1; -gW1]^T @ [ones; mu*rstd]   (rank-2 fixup)
     with Wg = g*W, gW1 = g@W, c1 = ln_b@W + bias (host-precomputed).
  2. Q^T,K^T in [head*dk, S] layout; V in [S, head*dk] layout, augmented
     with a ones column so attn@V_aug also yields the softmax denominator Z.
  3. Per head: scores^T matmul -> exp on ACT (no max subtraction;
     |scores/8| is small so exp is safe) -> AV matmul with V_aug
     stationary -> divide by Z (reciprocal + PE ones-outer broadcast).
     Odd heads use a column-shifted V_aug so every elementwise op stays
     lane-aligned (DVE cannot shift partitions).
  4. y^T = wo^T @ out^T + bo + x (residual), DMA out as [C, S];
     host transposes to [S, C].

Matmul inputs use float32r (fast fp32 mode, 1 cycle/row vs 4 for fp32).
f32r operands must be *produced* rounded-to-f32r by an engine op, so
every matmul-feeding tile is declared f32r and written by DVE/ACT.
Attention E/V tiles are bf16.
"""

import os
import sys

import numpy as np

for _p in ("/opt/trn_rl_repo",):
    if _p not in sys.path and os.path.isdir(_p):
        sys.path.insert(0, _p)

B, C, H, W = 8, 512, 32, 32
S = H * W  # 1024
NH, DK = 8, 64
HD = NH * DK  # 512
EPS = 1e-5
P = 128
NCORES = 8
KC = C // P  # 4 contraction chunks over channels
MC = HD // P  # 4 chunks over head*dk
ST = S // P  # 8 seq tiles
QH = S // 512  # 2 moving-dim halves

# matmul input dtype mode: "f32r" (fast fp32) or "f32" (exact, 4x slower)
MM_MODE = os.environ.get("BASS_MM_MODE", "f32r")


def _build_program():
    from concourse import bacc
    import concourse.mybir as mybir
    import concourse.tile as tile

    f32 = mybir.dt.float32
    bf16 = mybir.dt.bfloat16
    mmdt = mybir.dt.float32r if MM_MODE == "f32r" else f32
    Exp = mybir.ActivationFunctionType.Exp
    Sqrt = mybir.ActivationFunctionType.Sqrt

    nc = bacc.Bacc("TRN2", debug=False)

    x_d = nc.dram_tensor("x", [C, S], f32, kind="ExternalInput")
    wq_d = nc.dram_tensor("wq", [C, HD], f32, kind="ExternalInput")
    wk_d = nc.dram_tensor("wk", [C, HD], f32, kind="ExternalInput")
    wv_d = nc.dram_tensor("wv", [C, HD], f32, kind="ExternalInput")
    wo_d = nc.dram_tensor("wo", [HD, C], f32, kind="ExternalInput")
    fixq_d = nc.dram_tensor("fixq", [2, HD], f32, kind="ExternalInput")
    fixk_d = nc.dram_tensor("fixk", [2, HD], f32, kind="ExternalInput")
    fixv_d = nc.dram_tensor("fixv", [2, HD], f32, kind="ExternalInput")
    fixo_d = nc.dram_tensor("fixo", [1, C], f32, kind="ExternalInput")
    ident_d = nc.dram_tensor("ident", [P, P], f32, kind="ExternalInput")
    y_d = nc.dram_tensor("y", [C, S], f32, kind="ExternalOutput")

    with tile.TileContext(nc) as tc:
        with (
            tc.tile_pool(name="const", bufs=1) as const,
            tc.tile_pool(name="big", bufs=1) as big,
            tc.tile_pool(name="work", bufs=3) as work,
            tc.tile_pool(name="stg", bufs=2) as stg,
            tc.tile_pool(name="etp", bufs=6) as etp,
            tc.tile_pool(name="zp", bufs=2) as zp,
            tc.tile_pool(name="yp", bufs=2) as yp,
            tc.tile_pool(name="ps1", bufs=2, space="PSUM") as ps1,
            tc.tile_pool(name="ps2", bufs=3, space="PSUM") as ps2,
        ):
            # ---- constants / weights (DMA f32 staging -> round to mmdt) --
            ident = const.tile([P, P], f32, tag="ident")
            nc.sync.dma_start(ident, ident_d[:])

            w_sbs = {}
            for name, wd in (("wq", wq_d), ("wk", wk_d), ("wv", wv_d), ("wo", wo_d)):
                w_stg = stg.tile([P, KC, HD], f32, tag="wstg")
                nc.sync.dma_start(w_stg, wd.rearrange("(k p) n -> p k n", p=P))
                w_sbs[name] = const.tile([P, KC, HD], mmdt, tag=name, name=name + "_sb")
                nc.vector.tensor_copy(w_sbs[name], w_stg)
            wq_sb, wk_sb, wv_sb, wo_sb = (
                w_sbs["wq"], w_sbs["wk"], w_sbs["wv"], w_sbs["wo"],
            )

            fix_sbs = {}
            for name, fd, rows in (
                ("fixq", fixq_d, 2), ("fixk", fixk_d, 2),
                ("fixv", fixv_d, 2), ("fixo", fixo_d, 1),
            ):
                f_stg = stg.tile([2, HD], f32, tag="fstg")
                nc.sync.dma_start(f_stg[0:rows, :], fd[:])
                fix_sbs[name] = const.tile([2, HD], mmdt, tag=name, name=name + "_sb")
                nc.vector.tensor_copy(fix_sbs[name][0:rows, :], f_stg[0:rows, :])
            fixq_sb, fixk_sb, fixv_sb, fixo_sb = (
                fix_sbs["fixq"], fix_sbs["fixk"], fix_sbs["fixv"], fix_sbs["fixo"],
            )

            # f32r tiles cannot be memset directly; build masks in f32
            # staging and round via tensor_copy.
            zstg = stg.tile([P, P], f32, tag="zstg")
            nc.vector.memset(zstg, 0.0)
            nc.vector.memset(zstg[64:65, 0:DK], 1.0)
            nc.vector.memset(zstg[32:33, DK:P], 1.0)
            zmask = const.tile([P, P], mmdt, tag="zmask")
            nc.vector.tensor_copy(zmask, zstg)
            ones_stg = stg.tile([1, P], f32, tag="ones_stg")
            nc.vector.memset(ones_stg, 1.0)
            ones_c = const.tile([1, P], mmdt, tag="ones_c")
            nc.vector.tensor_copy(ones_c, ones_stg)

            xsb = big.tile([P, KC, S], f32, tag="xsb")
            nc.sync.dma_start(xsb, x_d.rearrange("(k p) s -> p k s", p=P))

            # ---- stats: mean/var per seq position ------------------------
            with nc.named_scope("stats"):
                mv = const.tile([P, ST, 2], f32, tag="mv")
                for st in range(ST):
                    pxt = ps1.tile([P, 512], f32, tag="ps1")
                    for k in range(KC):
                        nc.tensor.transpose(
                            pxt[:, k * P : (k + 1) * P],
                            xsb[:, k, st * P : (st + 1) * P],
                            ident,
                        )
                    st6 = work.tile([P, 6], f32, tag="st6")
                    nc.vector.bn_stats(st6, pxt)
                    nc.vector.bn_aggr(mv[:, st, :], st6)
                eps_ap = const.tile([P, 1], f32, tag="eps")
                nc.vector.memset(eps_ap, EPS)
                sd = work.tile([P, ST], f32, tag="sd")
                nc.scalar.activation(sd, mv[:, :, 1], Sqrt, bias=eps_ap)
                pkr = const.tile([P, ST, 1], f32, tag="pkr")
                pkf = const.tile([P, ST, 2], f32, tag="pkf")
                nc.vector.memset(pkf[:, :, 0:1], 1.0)
                nc.vector.reciprocal(pkr[:, :, 0], sd)  # rstd
                nc.vector.tensor_mul(pkf[:, :, 1], mv[:, :, 0], pkr[:, :, 0])  # mu*rstd

                # transpose stat columns into row vectors over S; engine ops
                # need 32-aligned partition bases, so [ones; m2] transposes
                # as an adjacent pair for a single base-0 copy into fix_rhs.
                statsR = const.tile([1, S], mmdt, tag="statsR")
                fix_rhs = const.tile([2, S], mmdt, tag="fix_rhs")
                for g2 in range(2):
                    ppk_r = ps1.tile([P, 512], f32, tag="ps1", name="ppk_r")
                    ppk_f = ps1.tile([P, 512], f32, tag="ps1", name="ppk_f")
                    for st4 in range(4):
                        st = g2 * 4 + st4
                        nc.tensor.transpose(
                            ppk_r[0:1, st4 * P : (st4 + 1) * P], pkr[:, st, :], ident
                        )
                        nc.tensor.transpose(
                            ppk_f[0:2, st4 * P : (st4 + 1) * P], pkf[:, st, :], ident
                        )
                    nc.vector.tensor_copy(
                        statsR[0:1, g2 * 512 : (g2 + 1) * 512], ppk_r[0:1, 0:512]
                    )
                    nc.vector.tensor_copy(
                        fix_rhs[0:2, g2 * 512 : (g2 + 1) * 512], ppk_f[0:2, 0:512]
                    )

                # broadcast rstd over 128 partitions via ones outer product
                psum_R = ps2.tile([P, 2, 512], f32, tag="ps2")
                for n in range(QH):
                    nc.tensor.matmul(
                        psum_R[:, n, :],
                        ones_c[0:1, :],
                        statsR[0:1, n * 512 : (n + 1) * 512],
                        start=True,
                        stop=True,
                    )
                xt = big.tile([P, KC, S], mmdt, tag="xt")
                for k in range(KC):
                    nc.vector.tensor_mul(
                        xt[:, k, :],
                        xsb[:, k, :],
                        psum_R[:, :, :].rearrange("p a b -> p (a b)"),
                    )

            # ---- QKV projections ----------------------------------------
            qt_sb = big.tile([P, MC, S], mmdt, tag="qt")
            kt_sb = big.tile([P, MC, S], mmdt, tag="kt")
            vaug = big.tile([P, ST, NH, P], bf16, tag="vaug")
            with nc.named_scope("qkv"):
                nc.gpsimd.memset(vaug, 0.0)
                nc.vector.memset(vaug[:, :, 1::2, 32:33], 1.0)
                nc.vector.memset(vaug[:, :, 0::2, 64:65], 1.0)
                for w_sb, fix_sb, dst in (
                    (wq_sb, fixq_sb, qt_sb),
                    (wk_sb, fixk_sb, kt_sb),
                ):
                    for m in range(MC):
                        pq = ps2.tile([P, 2, 512], f32, tag="ps2")
                        for n in range(QH):
                            for k in range(KC):
                                nc.tensor.matmul(
                                    pq[:, n, :],
                                    w_sb[:, k, m * P : (m + 1) * P],
                                    xt[:, k, n * 512 : (n + 1) * 512],
                                    start=(k == 0),
                                    stop=False,
                                )
                            nc.tensor.matmul(
                                pq[:, n, :],
                                fix_sb[:, m * P : (m + 1) * P],
                                fix_rhs[:, n * 512 : (n + 1) * 512],
                                start=False,
                                stop=True,
                            )
                        nc.vector.tensor_copy(
                            dst[:, m, :], pq[:, :, :].rearrange("p a b -> p (a b)")
                        )
                for st in range(ST):
                    pv = ps1.tile([P, 512], f32, tag="ps1")
                    for k in range(KC):
                        nc.tensor.matmul(
                            pv,
                            xt[:, k, st * P : (st + 1) * P],
                            wv_sb[:, k, :],
                            start=(k == 0),
                            stop=False,
                        )
                    nc.tensor.matmul(
                        pv,
                        fix_rhs[:, st * P : (st + 1) * P],
                        fixv_sb,
                        start=False,
                        stop=True,
                    )
                    pvv = pv.rearrange("p (h d) -> p h d", h=NH)
                    nc.vector.tensor_copy(vaug[:, st, 0::2, 0:64], pvv[:, 0::2, :])
                    nc.vector.tensor_copy(vaug[:, st, 1::2, 64:128], pvv[:, 1::2, :])

            # ---- attention ----------------------------------------------
            ot_sb = big.tile([P, MC, S], mmdt, tag="ot")
            with nc.named_scope("attn"):
                for h in range(NH):
                    par = h % 2
                    off = par * 64
                    hc = h // 2
                    pav = [ps1.tile([P, 512], f32, tag="ps1", name=f"pav{h}_{i}") for i in range(QH)]
                    for kt in range(ST):
                        pscore = ps2.tile([P, 2, 512], f32, tag="ps2")
                        for n in range(QH):
                            nc.tensor.matmul(
                                pscore[:, n, :],
                                kt_sb[off : off + 64, hc, kt * P : (kt + 1) * P],
                                qt_sb[off : off + 64, hc, n * 512 : (n + 1) * 512],
                                start=True,
                                stop=True,
                            )
                        et = etp.tile([P, S], bf16, tag="et")
                        nc.scalar.activation(
                            et,
                            pscore[:, :, :].rearrange("p a b -> p (a b)"),
                            Exp,
                            scale=0.125,
                        )
                        for n in range(QH):
                            if par == 0:
                                out_ap = pav[n][0:65, :]
                                lhs_ap = vaug[:, kt, h, 0:65]
                            else:
                                out_ap = pav[n][0:128, :]
                                lhs_ap = vaug[:, kt, h, 0:128]
                            nc.tensor.matmul(
                                out_ap,
                                lhs_ap,
                                et[:, n * 512 : (n + 1) * 512],
                                start=(kt == 0),
                                stop=(kt == ST - 1),
                            )
                    zbase = 64 if par == 0 else 32
                    obase = 0 if par == 0 else 64
                    zrow = zp.tile([P, 2, 512], mmdt, tag="zrow")
                    zb = zp.tile([P, 2, 512], f32, tag="zb")
                    pz = ps2.tile([P, 2, 512], f32, tag="ps2")
                    for n in range(QH):
                        nc.vector.tensor_copy(
                            zrow[zbase : zbase + 1, n, :], pav[n][zbase : zbase + 1, :]
                        )
                        with nc.allow_low_precision(
                            reason="1/Z stored f32r for the broadcast matmul; "
                            "f32r has ample precision for softmax scaling"
                        ):
                            nc.vector.reciprocal(
                                zrow[zbase : zbase + 1, n, :],
                                zrow[zbase : zbase + 1, n, :],
                            )
                        if par == 0:
                            nc.tensor.matmul(
                                pz[0:64, n, :],
                                zmask[64:65, 0:DK],
                                zrow[64:65, n, :],
                                start=True,
                                stop=True,
                            )
                        else:
                            nc.tensor.matmul(
                                pz[0:128, n, :],
                                zmask[32:33, 0:P],
                                zrow[32:33, n, :],
                                start=True,
                                stop=True,
                            )
                        nc.vector.tensor_copy(
                            zb[obase : obase + 64, n, :], pz[obase : obase + 64, n, :]
                        )
                        nc.vector.tensor_mul(
                            ot_sb[obase : obase + 64, hc, n * 512 : (n + 1) * 512],
                            pav[n][obase : obase + 64, :],
                            zb[obase : obase + 64, n, :],
                        )

            # ---- output projection + residual ---------------------------
            with nc.named_scope("out"):
                for m in range(MC):
                    py = ps2.tile([P, 2, 512], f32, tag="ps2")
                    for n in range(QH):
                        for k in range(MC):
                            nc.tensor.matmul(
                                py[:, n, :],
                                wo_sb[:, k, m * P : (m + 1) * P],
                                ot_sb[:, k, n * 512 : (n + 1) * 512],
                                start=(k == 0),
                                stop=False,
                            )
                        nc.tensor.matmul(
                            py[:, n, :],
                            fixo_sb[0:1, m * P : (m + 1) * P],
                            fix_rhs[0:1, n * 512 : (n + 1) * 512],
                            start=False,
                            stop=True,
                        )
                    yo = yp.tile([P, S], f32, tag="yo")
                    nc.vector.tensor_add(
                        yo, py[:, :, :].rearrange("p a b -> p (a b)"), xsb[:, m, :]
                    )
                    nc.sync.dma_start(y_d[m * P : (m + 1) * P, :], yo)
    nc.compile()
    return nc


_NC_CACHE = {}


def _get_program():
    key = MM_MODE
    if key not in _NC_CACHE:
        _NC_CACHE[key] = _build_program()
    return _NC_CACHE[key]


def _prepare_in_maps(x, wq, bq, wk, bk, wv, bv, ln_g, ln_b, wo, bo):
    f = np.float32
    x3 = np.ascontiguousarray(np.asarray(x, f).reshape(B, C, S))
    g = np.asarray(ln_g, f)
    b_ = np.asarray(ln_b, f)

    def fold(w, bias):
        w = np.asarray(w, f)
        wg = np.ascontiguousarray(g[:, None] * w)
        fix = np.ascontiguousarray(
            np.stack([b_ @ w + np.asarray(bias, f), -(g @ w)]).astype(f)
        )
        return wg, fix

    wqg, fixq = fold(wq, bq)
    wkg, fixk = fold(wk, bk)
    wvg, fixv = fold(wv, bv)
    shared = {
        "wq": wqg,
        "wk": wkg,
        "wv": wvg,
        "wo": np.ascontiguousarray(np.asarray(wo, f)),
        "fixq": fixq,
        "fixk": fixk,
        "fixv": fixv,
        "fixo": np.ascontiguousarray(np.asarray(bo, f).reshape(1, C)),
        "ident": np.eye(P, dtype=f),
    }
    return [{"x": np.ascontiguousarray(x3[i]), **shared} for i in range(NCORES)]


def run(inputs: dict, trace: bool = False, **run_kwargs):
    from concourse.bass_utils import run_bass_kernel_spmd

    nc = _get_program()
    in_maps = _prepare_in_maps(**inputs)
    res = run_bass_kernel_spmd(
        nc, in_maps, core_ids=list(range(NCORES)), trace=trace, **run_kwargs
    )
    y = np.stack([res.results[i]["y"] for i in range(NCORES)])  # [B, C, S]
    out = np.ascontiguousarray(y.transpose(0, 2, 1))  # [B, S, C]
    return out, res


def kernel(**inputs) -> np.ndarray:
    out, _ = run(inputs, trace=False)
    return out
